# revision 1
# baseline (speedup 1.0000x reference)
"""Trainium2 Bass kernel for GraphTransformerNet (star-graph TransformerConv).

Shapes (hardcoded): B=1024 graphs, N=128 neighbors, D=256 in-dim,
H=4 heads x C=64 = F=256 out-dim. Data-parallel over 8 NeuronCores
(128 graphs/core). Host pre-transposes inputs to [B, D, N] bf16 so the
contraction dim lands on SBUF partitions with zero on-chip transposes.
The tiny q projection (0.05% of FLOPs) is done host-side and shipped as
pre-masked "Qblock" columns; the central skip projection seeds the
aggregation PSUM accumulator so no final add is needed.

Per graph g (per core):
  kT[f,n] = Wk.T @ xT_g + We.T @ eT_g          (PSUM-accumulated, batched x4 graphs)
  v[n,f]  = xT_g.T @ Wv + eT_g.T @ We          } one [128,512] psum: [v | skip_n]
  skip[n,f] = xT_g.T @ Wskip                   }
  scoresT[n,h] = kT_chunk.T @ Qblock_chunk     (2 matmuls, 2 cols each)
  softmax over n: packed 8 graphs -> [32,128] rows=(g,h), PE-transposed
  agg_ps[f_loc,fc,g] (+)= v_slice.T @ alphaT_col  (4 tiny matmuls/graph,
        accumulated on top of Wskip.T @ central seeded at start)
  central row = agg_ps transposed back at the end
"""

import sys

import numpy as np

for _p in ("/opt/trn_rl_repo",):
    if _p not in sys.path:
        sys.path.insert(0, _p)

import ml_dtypes

import concourse.bacc as bacc
import concourse.bass as bass
import concourse.mybir as mybir
from concourse.bass import MemorySpace
from concourse.tile import TileContext

BF16 = mybir.dt.bfloat16
F32 = mybir.dt.float32
AFT = mybir.ActivationFunctionType

B, N, D, H, C = 1024, 128, 256, 4, 64
F = H * C            # 256
NCORES = 8
BG = B // NCORES     # 128 graphs per core
GROUP = 8            # graphs per softmax pack
NB4 = 4              # graphs per kT matmul batch
ROWS = N + 1         # 129 output rows per graph

_cached = {}


def _build_nc():
    nc = bacc.Bacc()

    xt_d = nc.dram_tensor("xt", [BG, D, N], BF16, kind="ExternalInput")
    et_d = nc.dram_tensor("et", [BG, D, N], BF16, kind="ExternalInput")
    ct_d = nc.dram_tensor("ct", [D, BG], BF16, kind="ExternalInput")
    qb0_d = nc.dram_tensor("qb0", [128, BG, 2], BF16, kind="ExternalInput")
    qb1_d = nc.dram_tensor("qb1", [128, BG, 2], BF16, kind="ExternalInput")
    wk_d = nc.dram_tensor("wk", [D, F], BF16, kind="ExternalInput")
    we_d = nc.dram_tensor("we", [D, F], BF16, kind="ExternalInput")
    wvs_d = nc.dram_tensor("wvs", [D, 2 * F], BF16, kind="ExternalInput")
    idb_d = nc.dram_tensor("idb", [128, 128], BF16, kind="ExternalInput")
    idf_d = nc.dram_tensor("idf", [128, 128], F32, kind="ExternalInput")
    out_d = nc.dram_tensor("out", [BG * ROWS, F], F32, kind="ExternalOutput")

    out_rows = out_d[:, :].rearrange("(g r) f -> g r f", r=ROWS)

    with TileContext(nc) as tc:
        with (
            tc.tile_pool(name="consts", bufs=1) as consts,
            tc.tile_pool(name="io", bufs=8) as io,
            tc.tile_pool(name="ktsb", bufs=4) as ktsb_pool,
            tc.tile_pool(name="vsb", bufs=2 * GROUP + 4) as vsb_pool,
            tc.tile_pool(name="skipsb", bufs=4) as skip_pool,
            tc.tile_pool(name="misc", bufs=10) as misc,
            tc.tile_pool(name="kt_ps", bufs=2, space=MemorySpace.PSUM) as kt_psp,
            tc.tile_pool(name="vs_ps", bufs=2, space=MemorySpace.PSUM) as vs_psp,
            tc.tile_pool(name="sps", bufs=3, space=MemorySpace.PSUM) as sps,
            tc.tile_pool(name="agg_ps", bufs=1, space=MemorySpace.PSUM) as agg_psp,
        ):
            # ---- constants ----
            wk_sb, we_sb, wvs_sb, ct_sb = [], [], [], []
            for dc in range(2):
                dsl = slice(dc * 128, (dc + 1) * 128)
                t = consts.tile([128, F], BF16, tag=f"wk{dc}")
                nc.sync.dma_start(t[:, :], wk_d[dsl, :]); wk_sb.append(t)
                t = consts.tile([128, F], BF16, tag=f"we{dc}")
                nc.sync.dma_start(t[:, :], we_d[dsl, :]); we_sb.append(t)
                t = consts.tile([128, 2 * F], BF16, tag=f"wvs{dc}")
                nc.sync.dma_start(t[:, :], wvs_d[dsl, :]); wvs_sb.append(t)
                t = consts.tile([128, BG], BF16, tag=f"ct{dc}")
                nc.sync.dma_start(t[:, :], ct_d[dsl, :]); ct_sb.append(t)
            qb_sb = []
            for fc, qb_d in enumerate((qb0_d, qb1_d)):
                t = consts.tile([128, BG, 2], BF16, tag=f"qb{fc}")
                nc.sync.dma_start(t[:, :, :], qb_d[:, :, :])
                qb_sb.append(t)
            idb = consts.tile([128, 128], BF16, tag="idb")
            nc.sync.dma_start(idb[:, :], idb_d[:, :])
            idf = consts.tile([128, 128], F32, tag="idf")
            nc.sync.dma_start(idf[:, :], idf_d[:, :])

            # aggregated messages, transposed layout [f_loc, fc, g]; held all
            # kernel. Seeded with the central skip projection Wskip.T @ cT so
            # the per-graph agg matmuls accumulate the final central rows.
            # start=True only on the very first matmul: it clears has_written
            # for the WHOLE bank, so any later start=True here would wipe the
            # other chunk's bits and break accumulation (overwrite semantics).
            agg_ps = agg_psp.tile([128, 2, BG], F32, tag="agg")
            for fc in range(2):
                fsl = slice(F + fc * 128, F + (fc + 1) * 128)
                for dc in range(2):
                    nc.tensor.matmul(agg_ps[:, fc, :], wvs_sb[dc][:, fsl],
                                     ct_sb[dc][:, :],
                                     start=(fc == 0 and dc == 0), stop=False,
                                     skip_group_check=True)

            # ---- main loop over groups of 8 graphs ----
            for grp in range(BG // GROUP):
                g0 = grp * GROUP
                xt8, et8 = [], []
                for dc in range(2):
                    dsl = slice(dc * 128, (dc + 1) * 128)
                    t = io.tile([128, GROUP, N], BF16, tag=f"xt{dc}")
                    nc.sync.dma_start(t[:, :, :],
                                      xt_d[g0:g0 + GROUP, dsl, :].rearrange("g d n -> d g n"))
                    xt8.append(t)
                    t = io.tile([128, GROUP, N], BF16, tag=f"et{dc}")
                    nc.sync.dma_start(t[:, :, :],
                                      et_d[g0:g0 + GROUP, dsl, :].rearrange("g d n -> d g n"))
                    et8.append(t)

                scoresT_ps = sps.tile([128, GROUP * 4], F32, tag="sps")
                v_tiles = []

                for b4 in range(GROUP // NB4):
                    bsl = slice(b4 * NB4, (b4 + 1) * NB4)
                    kt_sb = []
                    for fc in range(2):
                        fsl = slice(fc * 128, (fc + 1) * 128)
                        kt_ps = kt_psp.tile([128, NB4, N], F32, tag="ktps")
                        nc.tensor.matmul(kt_ps[:, :, :], wk_sb[0][:, fsl], xt8[0][:, bsl, :], start=True, stop=False)
                        nc.tensor.matmul(kt_ps[:, :, :], wk_sb[1][:, fsl], xt8[1][:, bsl, :], start=False, stop=False)
                        nc.tensor.matmul(kt_ps[:, :, :], we_sb[0][:, fsl], et8[0][:, bsl, :], start=False, stop=False)
                        nc.tensor.matmul(kt_ps[:, :, :], we_sb[1][:, fsl], et8[1][:, bsl, :], start=False, stop=True)
                        kt = ktsb_pool.tile([128, NB4, N], BF16, tag="ktsb")
                        nc.vector.tensor_copy(kt[:, :, :], kt_ps[:, :, :])
                        kt_sb.append(kt)

                    for gl in range(NB4):
                        gg = b4 * NB4 + gl          # graph index within group
                        g = g0 + gg                 # graph index within core
                        # scoresT: [128 n, 2] per f-chunk
                        for fc in range(2):
                            nc.tensor.matmul(
                                scoresT_ps[:, gg * 4 + fc * 2: gg * 4 + fc * 2 + 2],
                                kt_sb[fc][:, gl, :], qb_sb[fc][:, g, :],
                                start=True, stop=True)
                        # v (+edge) and neighbor skip rows in one [128, 512] psum
                        vs_ps = vs_psp.tile([128, 2 * F], F32, tag="vsps")
                        nc.tensor.matmul(vs_ps[:, :], xt8[0][:, gg, :], wvs_sb[0][:, :], start=True, stop=False)
                        nc.tensor.matmul(vs_ps[:, :], xt8[1][:, gg, :], wvs_sb[1][:, :], start=False, stop=False)
                        nc.tensor.matmul(vs_ps[:, 0:F], et8[0][:, gg, :], we_sb[0][:, :],
                                         start=False, stop=False, skip_group_check=True)
                        nc.tensor.matmul(vs_ps[:, 0:F], et8[1][:, gg, :], we_sb[1][:, :],
                                         start=False, stop=True, skip_group_check=True)
                        v_sb = vsb_pool.tile([128, F], BF16, tag="vsb")
                        nc.vector.tensor_copy(v_sb[:, :], vs_ps[:, 0:F])
                        v_tiles.append(v_sb)
                        skip_sb = skip_pool.tile([128, F], F32, tag="skipsb")
                        # tiny prefix write: absorbs the WAR-on-out-DMA wait so
                        # the big copy below carries only the PE wait (ISA
                        # allows one sync wait per instruction)
                        nc.scalar.activation(skip_sb[0:1, 0:1], idf[0:1, 0:1], AFT.Copy)
                        nc.scalar.activation(skip_sb[:, :], vs_ps[:, F:2 * F], AFT.Copy)
                        nc.sync.dma_start(out_rows[g, 1:ROWS, :], skip_sb[:, :])

                # ---- packed softmax over the group: rows = (g_local, h) ----
                st_sb = misc.tile([128, GROUP * 4], F32, tag="stsb")
                nc.vector.tensor_copy(st_sb[:, :], scoresT_ps[:, :])
                strans_ps = sps.tile([GROUP * 4, 128], F32, tag="sps")
                nc.tensor.transpose(strans_ps[:, :], st_sb[:, :], idf[:, :])
                mx = misc.tile([GROUP * 4, 1], F32, tag="mx")
                nc.vector.reduce_max(mx[:, :], strans_ps[:, :], axis=mybir.AxisListType.X)
                nmx = misc.tile([GROUP * 4, 1], F32, tag="nmx")
                # negate on ScalarE so exp's bias dep is same-engine
                nc.scalar.activation(nmx[:, :], mx[:, :], AFT.Copy, scale=-1.0)
                alpha_sb = misc.tile([GROUP * 4, 128], BF16, tag="alpha")
                sumexp = misc.tile([GROUP * 4, 1], F32, tag="sumexp")
                nc.scalar.activation(alpha_sb[:, :], strans_ps[:, :], AFT.Exp,
                                     bias=nmx[:, 0:1], accum_out=sumexp[:, 0:1])
                rsum = misc.tile([GROUP * 4, 1], F32, tag="rsum")
                nc.vector.reciprocal(rsum[:, :], sumexp[:, :])
                nc.vector.tensor_scalar_mul(alpha_sb[:, :], alpha_sb[:, :], rsum[:, 0:1])
                alphaT_ps = sps.tile([128, GROUP * 4], BF16, tag="sps")
                nc.tensor.transpose(alphaT_ps[:, :], alpha_sb[:, :],
                                    idb[0:GROUP * 4, 0:GROUP * 4])
                alphaT_sb = misc.tile([128, GROUP * 4], BF16, tag="alphaT")
                nc.vector.tensor_copy(alphaT_sb[:, :], alphaT_ps[:, :])

                # ---- aggregate: 4 tiny matmuls per graph, accumulate on the
                # skip-seeded psum ----
                for gg in range(GROUP):
                    g = g0 + gg
                    v_sb = v_tiles[gg]
                    for fc in range(2):
                        for hh in range(2):
                            h = fc * 2 + hh
                            last = (g == BG - 1 and fc == 1 and hh == 1)
                            nc.tensor.matmul(
                                agg_ps[hh * 64:(hh + 1) * 64, fc, g:g + 1],
                                v_sb[:, fc * 128 + hh * 64: fc * 128 + (hh + 1) * 64],
                                alphaT_sb[:, gg * 4 + h: gg * 4 + h + 1],
                                start=False, stop=last, skip_group_check=True)

            # ---- central rows: transpose agg back to [g, f] ----
            cenT_sb = misc.tile([128, 2, BG], F32, tag="cenT")
            nc.vector.tensor_copy(cenT_sb[:, :, :], agg_ps[:, :, :])
            cen_sb = misc.tile([128, F], F32, tag="censb")
            for fc in range(2):
                ct_ps = sps.tile([128, 128], F32, tag="sps")
                nc.tensor.transpose(ct_ps[:, :], cenT_sb[:, fc, :], idf[:, :])
                nc.vector.tensor_copy(cen_sb[:, fc * 128:(fc + 1) * 128], ct_ps[:, :])
            nc.sync.dma_start(out_rows[:, 0, :], cen_sb[:, :])

    nc.compile()
    return nc


def kernel(**inputs):
    x = np.asarray(inputs["neighbor_node_features"], dtype=np.float32)   # [B, N, D]
    e = np.asarray(inputs["edge_features"], dtype=np.float32)            # [B, N, D]
    cen = np.asarray(inputs["central_node_features"], dtype=np.float32)  # [B, 1, D]
    Wq = np.asarray(inputs["Wq"], dtype=np.float32)
    Wk = np.asarray(inputs["Wk"], dtype=np.float32)
    Wv = np.asarray(inputs["Wv"], dtype=np.float32)
    We = np.asarray(inputs["We"], dtype=np.float32)
    Ws = np.asarray(inputs["Wskip"], dtype=np.float32)
    bq = np.asarray(inputs["bq"], dtype=np.float32)
    # biases are all zeros in this model family (bq folds into q host-side)
    for bn in ("bk", "bv", "bskip"):
        bv = np.asarray(inputs[bn])
        assert np.abs(bv).max() == 0.0, f"nonzero bias {bn} unsupported"

    bf = ml_dtypes.bfloat16
    xT = np.ascontiguousarray(x.transpose(0, 2, 1)).astype(bf)    # [B, D, N]
    eT = np.ascontiguousarray(e.transpose(0, 2, 1)).astype(bf)    # [B, D, N]
    cT = cen.reshape(B, D).T                                      # [D, B] f32
    wk = Wk.astype(bf)
    we = We.astype(bf)
    wvs = np.concatenate([Wv, Ws], axis=1).astype(bf)             # [D, 512]

    # host-side q projection + scaling + Qblock masking (tiny GEMM)
    qT = (Wq.T @ cT + bq[:, None]) * (1.0 / np.sqrt(C))           # [F, B] f32
    mask = (np.arange(128) // 64)[:, None] == np.arange(2)[None, :]   # [128, 2]
    qb = np.empty((2, 128, B, 2), dtype=np.float32)
    for fc in range(2):
        qb[fc] = qT[fc * 128:(fc + 1) * 128, :, None] * mask[:, None, :]
    qb = qb.astype(bf)

    idb = np.eye(128, dtype=np.float32).astype(bf)
    idf = np.eye(128, dtype=np.float32)

    if "nc" not in _cached:
        _cached["nc"] = _build_nc()
    nc = _cached["nc"]

    ctb = cT.astype(bf)
    in_maps = []
    for c in range(NCORES):
        gsl = slice(c * BG, (c + 1) * BG)
        in_maps.append({
            "xt": xT[gsl], "et": eT[gsl],
            "ct": np.ascontiguousarray(ctb[:, gsl]),
            "qb0": np.ascontiguousarray(qb[0][:, gsl]),
            "qb1": np.ascontiguousarray(qb[1][:, gsl]),
            "wk": wk, "we": we, "wvs": wvs,
            "idb": idb, "idf": idf,
        })

    from concourse.bass_utils import run_bass_kernel_spmd
    res = run_bass_kernel_spmd(nc, in_maps, core_ids=list(range(NCORES)),
                               **_cached.get("run_kwargs", {}))
    _cached["last_results"] = res
    out = np.concatenate([np.asarray(r["out"]) for r in res.results], axis=0)
    return out.astype(np.float32)



# revision 2
# speedup vs baseline: 1.5263x; 1.5263x over previous
"""Trainium2 Bass kernel for GraphTransformerNet (star-graph TransformerConv).

Shapes (hardcoded): B=1024 graphs, N=128 neighbors, D=256 in-dim,
H=4 heads x C=64 = F=256 out-dim. Data-parallel over 8 NeuronCores
(128 graphs/core).

Key structure (v2 — scores via host-folded q):
  The attention logits only need q.k = x @ (Wk q) + e @ (We q), so the
  per-graph q vector is folded into tiny per-graph weight columns
  wkq[d,g,h] host-side. That removes the whole kT projection pipeline
  (a third of PE streaming) and its PSUM->SBUF casts. Per graph the PE
  runs 4 big matmuls ([v|skip] from x, v from e; stationary = the
  graph's x/e d-chunk) plus 4 nearly-free 4-column score matmuls that
  reuse those stationaries.

  Softmax is max-free (scores ~ N(0,2), no overflow): Act exps the
  [128n, 32(g,h)] score block; the sums over n come from a ones-vector
  matmul; normalization happens on the host (agg and sums ship raw).
  Aggregation = 4 ap=1 matmuls per graph into a persistent PSUM tile
  [128 f_loc, 2 fc, BG], consumed one group behind the producer so the
  PE never waits on Act. Central skip projection runs once at the end
  into its own PSUM; host adds agg/sums to it in f32.

  Inputs ship as [D, BG, x|e] so each group needs just two 2D DMAs;
  skip rows buffer per-group and leave in one DMA as bf16 to an
  [N, BG, F] layout (host transposes back).
"""

import sys

import numpy as np

for _p in ("/opt/trn_rl_repo",):
    if _p not in sys.path:
        sys.path.insert(0, _p)

import ml_dtypes

import concourse.bacc as bacc
import concourse.bass as bass
import concourse.mybir as mybir
from concourse.bass import MemorySpace
from concourse.tile import TileContext

BF16 = mybir.dt.bfloat16
F32 = mybir.dt.float32
AFT = mybir.ActivationFunctionType

B, N, D, H, C = 1024, 128, 256, 4, 64
F = H * C            # 256
NCORES = 8
BG = B // NCORES     # 128 graphs per core
GROUP = 8            # graphs per group (softmax/DMA batch)
NG = BG // GROUP     # 16 groups
ROWS = N + 1         # 129 output rows per graph

_cached = {}


def _build_nc():
    nc = bacc.Bacc()

    xe_d = nc.dram_tensor("xe", [D, BG, 2 * N], BF16, kind="ExternalInput")
    wkq_d = nc.dram_tensor("wkq", [D, BG, H], BF16, kind="ExternalInput")
    weq_d = nc.dram_tensor("weq", [D, BG, H], BF16, kind="ExternalInput")
    wvs_d = nc.dram_tensor("wvs", [D, 2 * F], BF16, kind="ExternalInput")
    we_d = nc.dram_tensor("we", [D, F], BF16, kind="ExternalInput")
    ct_d = nc.dram_tensor("ct", [D, BG], BF16, kind="ExternalInput")
    ones_d = nc.dram_tensor("ones", [128, 1], BF16, kind="ExternalInput")

    skip_d = nc.dram_tensor("skip", [N, BG, F], BF16, kind="ExternalOutput")
    aggT_d = nc.dram_tensor("aggT", [128, 2, BG], F32, kind="ExternalOutput")
    skT_d = nc.dram_tensor("skT", [128, 2, BG], F32, kind="ExternalOutput")
    sums_d = nc.dram_tensor("sums", [1, BG * H], F32, kind="ExternalOutput")

    with TileContext(nc) as tc:
        with (
            tc.tile_pool(name="consts", bufs=1) as consts,
            tc.tile_pool(name="io", bufs=6) as io,
            tc.tile_pool(name="vsb", bufs=2 * GROUP + 2) as v_pool,
            tc.tile_pool(name="skipsb", bufs=2) as skip_pool,
            tc.tile_pool(name="expsb", bufs=3) as exp_pool,
            tc.tile_pool(name="misc", bufs=6) as misc,
            tc.tile_pool(name="vs_ps", bufs=3, space=MemorySpace.PSUM) as vs_psp,
            tc.tile_pool(name="sc_ps", bufs=2, space=MemorySpace.PSUM) as sc_psp,
            tc.tile_pool(name="agg_ps", bufs=1, space=MemorySpace.PSUM) as agg_psp,
            tc.tile_pool(name="sum_ps", bufs=1, space=MemorySpace.PSUM) as sum_psp,
            tc.tile_pool(name="skc_ps", bufs=1, space=MemorySpace.PSUM) as skc_psp,
        ):
            # ---- constants ----
            wvs_sb, we_sb, wkq_sb, weq_sb, ct_sb = [], [], [], [], []
            for dc in range(2):
                dsl = slice(dc * 128, (dc + 1) * 128)
                t = consts.tile([128, 2 * F], BF16, tag=f"wvs{dc}")
                nc.sync.dma_start(t[:, :], wvs_d[dsl, :]); wvs_sb.append(t)
                t = consts.tile([128, F], BF16, tag=f"we{dc}")
                nc.sync.dma_start(t[:, :], we_d[dsl, :]); we_sb.append(t)
                t = consts.tile([128, BG, H], BF16, tag=f"wkq{dc}")
                nc.sync.dma_start(t[:, :, :], wkq_d[dsl, :, :]); wkq_sb.append(t)
                t = consts.tile([128, BG, H], BF16, tag=f"weq{dc}")
                nc.sync.dma_start(t[:, :, :], weq_d[dsl, :, :]); weq_sb.append(t)
                t = consts.tile([128, BG], BF16, tag=f"ct{dc}")
                nc.sync.dma_start(t[:, :], ct_d[dsl, :]); ct_sb.append(t)
            ones_sb = consts.tile([128, 1], BF16, tag="ones")
            nc.sync.dma_start(ones_sb[:, :], ones_d[:, :])

            # persistent PSUM: unnormalized aggregated messages in
            # [f_loc, fc, g] layout, and per-(g,h) exp-sums. Each element
            # is written by exactly one matmul; start=True only on the
            # very first write into each bank (clears has_written for the
            # whole bank -> first touch of every address overwrites).
            agg_ps = agg_psp.tile([128, 2, BG], F32, tag="agg")
            sums_ps = sum_psp.tile([1, BG * H], F32, tag="sums")

            state = {}

            def consume(j):
                exp_sb, v_sbs = state.pop(j)
                nc.tensor.matmul(sums_ps[0:1, j * 32:(j + 1) * 32],
                                 ones_sb[:, :], exp_sb[:, :],
                                 start=(j == 0), stop=(j == NG - 1),
                                 skip_group_check=(j > 0))
                for gg in range(GROUP):
                    g = j * GROUP + gg
                    for fc in range(2):
                        for hh in range(2):
                            h = fc * 2 + hh
                            first = (g == 0 and fc == 0 and hh == 0)
                            last = (g == BG - 1 and fc == 1 and hh == 1)
                            nc.tensor.matmul(
                                agg_ps[hh * 64:(hh + 1) * 64, fc, g:g + 1],
                                v_sbs[gg][:, fc * 128 + hh * 64:
                                          fc * 128 + (hh + 1) * 64],
                                exp_sb[:, gg * 4 + h:gg * 4 + h + 1],
                                start=first, stop=last,
                                skip_group_check=not first)

            # ---- main loop over groups of 8 graphs ----
            for grp in range(NG):
                g0 = grp * GROUP
                xe = []
                for dc in range(2):
                    dsl = slice(dc * 128, (dc + 1) * 128)
                    t = io.tile([128, GROUP, 2 * N], BF16, tag=f"xe{dc}")
                    nc.sync.dma_start(t[:, :, :], xe_d[dsl, g0:g0 + GROUP, :])
                    xe.append(t)

                scoresT_ps = sc_psp.tile([128, GROUP * H], F32, tag="scps")
                skip_t = skip_pool.tile([128, GROUP, F], BF16, tag="skipsb")
                v_sbs = []

                for gg in range(GROUP):
                    g = g0 + gg
                    ssl = slice(gg * 4, gg * 4 + 4)
                    xs = [xe[0][:, gg, 0:N], xe[1][:, gg, 0:N]]
                    es = [xe[0][:, gg, N:2 * N], xe[1][:, gg, N:2 * N]]
                    vs_ps = vs_psp.tile([128, 2 * F], F32, tag="vsps")
                    nc.tensor.matmul(vs_ps[:, :], xs[0], wvs_sb[0][:, :],
                                     start=True, stop=False)
                    nc.tensor.matmul(scoresT_ps[:, ssl], xs[0],
                                     wkq_sb[0][:, g, :],
                                     start=(gg == 0), stop=False,
                                     skip_group_check=(gg > 0))
                    nc.tensor.matmul(vs_ps[:, :], xs[1], wvs_sb[1][:, :],
                                     start=False, stop=False)
                    nc.tensor.matmul(scoresT_ps[:, ssl], xs[1],
                                     wkq_sb[1][:, g, :],
                                     start=False, stop=False,
                                     skip_group_check=True)
                    nc.tensor.matmul(vs_ps[:, 0:F], es[0], we_sb[0][:, :],
                                     start=False, stop=False,
                                     skip_group_check=True)
                    nc.tensor.matmul(scoresT_ps[:, ssl], es[0],
                                     weq_sb[0][:, g, :],
                                     start=False, stop=False,
                                     skip_group_check=True)
                    nc.tensor.matmul(vs_ps[:, 0:F], es[1], we_sb[1][:, :],
                                     start=False, stop=True,
                                     skip_group_check=True)
                    nc.tensor.matmul(scoresT_ps[:, ssl], es[1],
                                     weq_sb[1][:, g, :],
                                     start=False, stop=(gg == GROUP - 1),
                                     skip_group_check=True)

                    v_sb = v_pool.tile([128, F], BF16, tag="vsb")
                    nc.vector.tensor_copy(v_sb[:, :], vs_ps[:, 0:F])
                    v_sbs.append(v_sb)
                    # tiny prefix write absorbs the WAR-on-out-DMA wait so
                    # the big copy carries only the PE wait
                    nc.scalar.activation(skip_t[0:1, gg, 0:1],
                                         ones_sb[0:1, 0:1], AFT.Copy)
                    nc.scalar.activation(skip_t[:, gg, :], vs_ps[:, F:2 * F],
                                         AFT.Copy)
                nc.sync.dma_start(skip_d[:, g0:g0 + GROUP, :], skip_t[:, :, :])

                exp_sb = exp_pool.tile([128, GROUP * H], BF16, tag="expsb")
                nc.scalar.activation(exp_sb[:, :], scoresT_ps[:, :], AFT.Exp)
                state[grp] = (exp_sb, v_sbs)
                if grp >= 1:
                    consume(grp - 1)

            consume(NG - 1)

            # ---- central skip projection: skT[f_loc, fc, g] ----
            skc_ps = skc_psp.tile([128, 2, BG], F32, tag="skc")
            for fc in range(2):
                fsl = slice(F + fc * 128, F + (fc + 1) * 128)
                for dc in range(2):
                    nc.tensor.matmul(skc_ps[:, fc, :], wvs_sb[dc][:, fsl],
                                     ct_sb[dc][:, :],
                                     start=(fc == 0 and dc == 0),
                                     stop=(fc == 1 and dc == 1),
                                     skip_group_check=(fc == 1))

            # ---- ship raw agg / skipcen / sums; host normalizes ----
            aggT_sb = misc.tile([128, 2, BG], F32, tag="aggT")
            nc.vector.tensor_copy(aggT_sb[:, :, :], agg_ps[:, :, :])
            nc.sync.dma_start(aggT_d[:, :, :], aggT_sb[:, :, :])
            skT_sb = misc.tile([128, 2, BG], F32, tag="skT")
            nc.scalar.activation(skT_sb[:, :, :], skc_ps[:, :, :], AFT.Copy)
            nc.sync.dma_start(skT_d[:, :, :], skT_sb[:, :, :])
            sums_sb = misc.tile([1, BG * H], F32, tag="sumsb")
            nc.vector.tensor_copy(sums_sb[:, :], sums_ps[:, :])
            nc.sync.dma_start(sums_d[:, :], sums_sb[:, :])

    nc.compile()
    return nc


def kernel(**inputs):
    x = np.asarray(inputs["neighbor_node_features"], dtype=np.float32)   # [B, N, D]
    e = np.asarray(inputs["edge_features"], dtype=np.float32)            # [B, N, D]
    cen = np.asarray(inputs["central_node_features"], dtype=np.float32)  # [B, 1, D]
    Wq = np.asarray(inputs["Wq"], dtype=np.float32)
    Wk = np.asarray(inputs["Wk"], dtype=np.float32)
    Wv = np.asarray(inputs["Wv"], dtype=np.float32)
    We = np.asarray(inputs["We"], dtype=np.float32)
    Ws = np.asarray(inputs["Wskip"], dtype=np.float32)
    bq = np.asarray(inputs["bq"], dtype=np.float32)
    # biases are all zeros in this model family (bq folds into q host-side)
    for bn in ("bk", "bv", "bskip"):
        bv = np.asarray(inputs[bn])
        assert np.abs(bv).max() == 0.0, f"nonzero bias {bn} unsupported"

    bf = ml_dtypes.bfloat16
    cT = cen.reshape(B, D).T                                      # [D, B] f32

    # host-side q projection + scaling + fold into per-graph weight columns
    qs = (Wq.T @ cT + bq[:, None]) * (1.0 / np.sqrt(C))           # [F, B]
    qs4 = qs.reshape(H, C, B)
    wkq = np.matmul(Wk.reshape(D, H, C).transpose(1, 0, 2), qs4)  # [H, D, B]
    weq = np.matmul(We.reshape(D, H, C).transpose(1, 0, 2), qs4)
    wkq = np.ascontiguousarray(wkq.transpose(1, 2, 0)).astype(bf)  # [D, B, H]
    weq = np.ascontiguousarray(weq.transpose(1, 2, 0)).astype(bf)

    wvs = np.concatenate([Wv, Ws], axis=1).astype(bf)             # [D, 512]
    web = We.astype(bf)
    ctb = cT.astype(bf)
    ones = np.ones((128, 1), dtype=np.float32).astype(bf)

    # [D, B, x|e] combined layout: one 2D DMA per (group, d-chunk)
    xe = np.empty((D, B, 2 * N), dtype=bf)
    xe[:, :, 0:N] = x.transpose(2, 0, 1)
    xe[:, :, N:2 * N] = e.transpose(2, 0, 1)

    if "nc" not in _cached:
        _cached["nc"] = _build_nc()
    nc = _cached["nc"]

    in_maps = []
    for c in range(NCORES):
        gsl = slice(c * BG, (c + 1) * BG)
        in_maps.append({
            "xe": np.ascontiguousarray(xe[:, gsl]),
            "wkq": np.ascontiguousarray(wkq[:, gsl]),
            "weq": np.ascontiguousarray(weq[:, gsl]),
            "wvs": wvs, "we": web,
            "ct": np.ascontiguousarray(ctb[:, gsl]),
            "ones": ones,
        })

    from concourse.bass_utils import run_bass_kernel_spmd
    res = run_bass_kernel_spmd(nc, in_maps, core_ids=list(range(NCORES)),
                               **_cached.get("run_kwargs", {}))
    _cached["last_results"] = res

    out = np.empty((B, ROWS, F), dtype=np.float32)
    for c, r in enumerate(res.results):
        gsl = slice(c * BG, (c + 1) * BG)
        skip = np.asarray(r["skip"]).astype(np.float32)       # [N, BG, F]
        out[gsl, 1:ROWS, :] = skip.transpose(1, 0, 2)
        aggT = np.asarray(r["aggT"])                          # [128, 2, BG]
        skT = np.asarray(r["skT"])
        s = np.asarray(r["sums"]).reshape(BG, H)              # [BG, H]
        agg = aggT.transpose(2, 1, 0).reshape(BG, F)          # [BG, 256]
        skc = skT.transpose(2, 1, 0).reshape(BG, F)
        out[gsl, 0, :] = skc + agg / np.repeat(s, C, axis=1)
    return out.reshape(B * ROWS, F)
